# revision 38
# baseline (speedup 1.0000x reference)
"""DilatedCNN forward on 8 TRN2 NeuronCores.

Strategy: data-parallel over the sequence dim N with halo. Each core owns
M=1024 rows plus an 8-row halo on each side (8 = sum of dilations
[1,2,4,1]); with the halo, all four layers are computed fully locally —
no collectives. The activation state lives in SBUF *transposed*
(feature-major: [128 partitions = feature chunk, rows in the free dim]) so
that
  * the concat [X, X_left, X_right] is just three column-shifted views of
    the same buffer (shifts along the free dim are free),
  * the 3072-feature contraction has features on partitions as the
    TensorEngine requires for both operands,
  * each layer's output is again feature-major — ready to be the next
    layer's input with no data movement,
  * the per-feature bias is a per-partition scalar for the activation op.
Matmuls run in float32r (TF32-path, full PE rate at free-dim >= 256); the
residual state stays fp32, with a rounded fp32r copy made per layer for
the GEMM inputs. Out-of-range rows are refreshed with the `oob` vector
between layers via copy_predicated driven by per-core mask/fill inputs,
so all 8 cores run one identical program.
"""

import numpy as np

import concourse.bacc as bacc
import concourse.mybir as mybir
import concourse.tile as tile
from concourse.bass_utils import run_bass_kernel_spmd

N, DIM, NL = 8192, 1024, 4
NCORES = 8
M = N // NCORES           # rows per core
H = 8                     # halo rows each side (sum of dilations)
PAD = 4                   # zero cols so shifted reads stay in-bounds
B = M + 2 * H             # 1040 buffer rows
FB = PAD + B + PAD        # 1048 free-dim cols of the state buffer
DIL = [1, 2, 4, 1]
KT = 3 * DIM // 128       # 24 contraction tiles
DT = DIM // 128           # 8 feature tiles
# Per-layer compute windows (rows [start, start+size) of the B-row buffer),
# shrinking by the dilation each layer; all sizes even (fp32r streams
# column pairs) and >= 256 (fp32r full-rate threshold).
ROW_BLOCKS_L = [
    [(1, 346), (347, 346), (693, 346)],   # layer 1: rows [1, 1039)
    [(3, 346), (349, 344), (693, 344)],   # layer 2: rows [3, 1037)
    [(7, 342), (349, 342), (691, 342)],   # layer 3: rows [7, 1033)
    [(8, 512), (520, 512)],               # layer 4: rows [8, 1032)
]
F32 = mybir.dt.float32
F32R = mybir.dt.float32r

_CACHE = {}
LAST_RESULTS = None  # test harness reads exec_time_ns from here


def _build():
    nc = bacc.Bacc("TRN2", target_bir_lowering=False, debug=False)

    xs_d = nc.dram_tensor("XST", [128, DT, B], F32, kind="ExternalInput")
    w_d = nc.dram_tensor("WT", [NL, 2, KT, 128, 512], F32, kind="ExternalInput")
    b_d = nc.dram_tensor("BS", [128, NL * DT], F32, kind="ExternalInput")
    ml_d = nc.dram_tensor("ML", [128, DT, H], mybir.dt.uint8, kind="ExternalInput")
    fl_d = nc.dram_tensor("FL", [128, DT, H], F32, kind="ExternalInput")
    mr_d = nc.dram_tensor("MR", [128, DT, H], mybir.dt.uint8, kind="ExternalInput")
    fr_d = nc.dram_tensor("FR", [128, DT, H], F32, kind="ExternalInput")
    y_d = nc.dram_tensor("YT", [128, DT, M], F32, kind="ExternalOutput")

    with tile.TileContext(nc) as tc:
        with (
            tc.tile_pool(name="state", bufs=1) as state_pool,
            tc.tile_pool(name="wpool", bufs=1) as w_pool,
            tc.tile_pool(name="const", bufs=1) as const_pool,
            tc.tile_pool(name="tmp", bufs=4) as tmp_pool,
            tc.tile_pool(name="gps", bufs=6, space="PSUM") as gps_pool,
        ):
            S = state_pool.tile([128, DT, FB], F32)    # fp32 residual state
            R = state_pool.tile([128, DT, FB], F32R)   # rounded GEMM input

            # zero the PAD columns once; epilogues never touch them
            nc.gpsimd.memset(S[:, :, 0:PAD], 0.0)
            nc.gpsimd.memset(S[:, :, PAD + B:FB], 0.0)

            # ---- entry: host pre-transposed X -> straight DMA into S ----
            # dt0 rides the fast sync queue (it gates the first matmul);
            # the rest go via GpSimd so the weight DMAs own sync
            for dt in range(DT):
                eng = nc.sync if dt == 0 else nc.gpsimd
                eng.dma_start(S[:, dt, PAD:PAD + B], xs_d[:, dt, :])

            bs_t = const_pool.tile([128, NL * DT], F32)
            mask_l = const_pool.tile([128, DT, H], mybir.dt.uint8)
            fill_l = const_pool.tile([128, DT, H], F32)
            mask_r = const_pool.tile([128, DT, H], mybir.dt.uint8)
            fill_r = const_pool.tile([128, DT, H], F32)
            nc.gpsimd.dma_start(bs_t[:], b_d[:])
            nc.gpsimd.dma_start(mask_l[:], ml_d[:])
            nc.gpsimd.dma_start(fill_l[:], fl_d[:])
            nc.gpsimd.dma_start(mask_r[:], mr_d[:])
            nc.gpsimd.dma_start(fill_r[:], fr_d[:])

            # ---- layers ----
            w_tiles = {}
            for l, d in enumerate(DIL):
                # rounded copy of the state for this layer's GEMMs
                for dt in range(DT):
                    nc.vector.tensor_copy(R[:, dt, :], S[:, dt, :])

                row_blocks = ROW_BLOCKS_L[l]
                for h in range(2):
                    for kt in range(KT):
                        wt = w_pool.tile([128, 512], F32R, tag=f"w{h}_{kt}")
                        w_tiles[(h, kt)] = wt
                        nc.sync.dma_start(
                            wt[:], w_d[l, h, kt].bitcast(F32R)
                        )
                    def mm_g(ps, c0, nb, mtl, kt):
                        dt = kt % DT
                        grp = kt // DT
                        sh = 0 if grp == 0 else (-d if grp == 1 else d)
                        nc.tensor.matmul(
                            ps[:, 0:nb],
                            w_tiles[(h, kt)][:, mtl * 128:(mtl + 1) * 128],
                            R[:, dt, PAD + c0 + sh:PAD + c0 + sh + nb],
                            start=(kt == 0),
                            stop=(kt == KT - 1),
                        )

                    def epilogue_g(ps, c0, nb, mtl):
                        mt = h * 4 + mtl
                        tmp = tmp_pool.tile([128, 512], F32, tag="tmp",
                                            name=f"tmp{l}_{h}_{c0}_{mtl}")
                        nc.scalar.activation(
                            tmp[:, 0:nb],
                            ps[:, 0:nb],
                            mybir.ActivationFunctionType.Relu,
                            bias=bs_t[:, l * DT + mt:l * DT + mt + 1],
                            scale=0.5,
                        )
                        # S = 0.5*S + relu(0.5*cat@W + 0.5*b), in place
                        nc.vector.scalar_tensor_tensor(
                            S[:, mt, PAD + c0:PAD + c0 + nb],
                            S[:, mt, PAD + c0:PAD + c0 + nb],
                            0.5,
                            tmp[:, 0:nb],
                            mybir.AluOpType.mult,
                            mybir.AluOpType.add,
                        )

                    groups = [(c0, nb, mtl)
                              for (c0, nb) in row_blocks
                              for mtl in range(4)]
                    # The very first half-layer races the weight DMAs (one
                    # k-tile lands per ~610ns but a single accumulation group
                    # consumes one per ~145ns). Interleave the first 6 groups
                    # per k-tile: consumption 6 x ~145ns/kt stays behind the
                    # supply, so the PE never waits on weights.
                    n_inter = 6 if (l == 0 and h == 0) else 0
                    head = groups[:n_inter]
                    if head:
                        pss = [
                            gps_pool.tile([128, 512], F32, tag="gps",
                                          name=f"psi{i}")
                            for i in range(len(head))
                        ]
                        for kt in range(KT):
                            for i, (c0, nb, mtl) in enumerate(head):
                                mm_g(pss[i], c0, nb, mtl, kt)
                        for i, (c0, nb, mtl) in enumerate(head):
                            epilogue_g(pss[i], c0, nb, mtl)
                    for j, (c0, nb, mtl) in enumerate(groups[n_inter:]):
                        ps = gps_pool.tile([128, 512], F32, tag="gps",
                                           name=f"ps{l}_{h}_{j}")
                        for kt in range(KT):
                            mm_g(ps, c0, nb, mtl, kt)
                        epilogue_g(ps, c0, nb, mtl)

                # refresh out-of-range halo rows with oob (data-driven; only
                # the edge cores have nonzero masks)
                if l < NL - 1:
                    for dt in range(DT):
                        nc.vector.copy_predicated(
                            S[:, dt, PAD:PAD + H],
                            mask_l[:, dt, :], fill_l[:, dt, :],
                        )
                        nc.vector.copy_predicated(
                            S[:, dt, PAD + B - H:PAD + B],
                            mask_r[:, dt, :], fill_r[:, dt, :],
                        )

            # ---- exit: dump the feature-major state; host untransposes ----
            # (two DMAs per dt so each half fires as soon as its last
            # epilogue lands, instead of waiting for the whole row range)
            for dt in range(DT):
                for c0h in (0, 512):
                    nc.sync.dma_start(
                        y_d[:, dt, c0h:c0h + 512],
                        S[:, dt, PAD + H + c0h:PAD + H + c0h + 512],
                    )

    nc.compile()
    return nc


def _get_nc():
    if "nc" not in _CACHE:
        _CACHE["nc"] = _build()
    return _CACHE["nc"]


def kernel(X, Ws, bs, oob):
    global LAST_RESULTS
    X = np.ascontiguousarray(np.asarray(X, np.float32))
    Ws = np.ascontiguousarray(np.asarray(Ws, np.float32))
    bs = np.ascontiguousarray(np.asarray(bs, np.float32))
    oob = np.ascontiguousarray(np.asarray(oob, np.float32))

    nc = _get_nc()

    # host-side input prep (pure layout rearrangement)
    WT = np.ascontiguousarray(
        Ws.reshape(NL, KT, 128, 2, 512).transpose(0, 3, 1, 2, 4)
    )
    BS = np.ascontiguousarray(
        (0.5 * bs).reshape(NL, DT, 128).transpose(2, 0, 1).reshape(128, NL * DT)
    )
    oobT = np.ascontiguousarray(oob.reshape(DT, 128).T)  # [128, DT]
    fill_edge = np.repeat(oobT[:, :, None], H, axis=2)   # [128, DT, H]
    ones = np.ones((128, DT, H), np.uint8)
    zeros_m = np.zeros((128, DT, H), np.uint8)
    zeros = np.zeros((128, DT, H), np.float32)

    in_maps = []
    for c in range(NCORES):
        lo, hi = c * M - H, c * M + M + H
        xs = np.empty((B, DIM), np.float32)
        slo, shi = max(lo, 0), min(hi, N)
        xs[slo - lo:shi - lo] = X[slo:shi]
        if lo < 0:
            xs[0:-lo] = oob
        if hi > N:
            xs[B - (hi - N):] = oob
        xst = np.ascontiguousarray(
            xs.reshape(B, DT, 128).transpose(2, 1, 0))
        left_edge = c == 0
        right_edge = c == NCORES - 1
        in_maps.append({
            "XST": xst,
            "WT": WT,
            "BS": BS,
            "ML": ones if left_edge else zeros_m,
            "FL": fill_edge if left_edge else zeros,
            "MR": ones if right_edge else zeros_m,
            "FR": fill_edge if right_edge else zeros,
        })

    res = run_bass_kernel_spmd(nc, in_maps, list(range(NCORES)))
    LAST_RESULTS = res
    out = np.concatenate(
        [res.results[c]["YT"].transpose(2, 1, 0).reshape(M, DIM)
         for c in range(NCORES)],
        axis=0,
    )
    return out[None, :, :].astype(np.float32)


# revision 41
# speedup vs baseline: 1.0067x; 1.0067x over previous
"""DilatedCNN forward on 8 TRN2 NeuronCores.

Strategy: data-parallel over the sequence dim N with halo. Each core owns
M=1024 rows plus an 8-row halo on each side (8 = sum of dilations
[1,2,4,1]); with the halo, all four layers are computed fully locally —
no collectives. The activation state lives in SBUF *transposed*
(feature-major: [128 partitions = feature chunk, rows in the free dim]) so
that
  * the concat [X, X_left, X_right] is just three column-shifted views of
    the same buffer (shifts along the free dim are free),
  * the 3072-feature contraction has features on partitions as the
    TensorEngine requires for both operands,
  * each layer's output is again feature-major — ready to be the next
    layer's input with no data movement,
  * the per-feature bias is a per-partition scalar for the activation op.
Matmuls run in float32r (TF32-path, full PE rate at free-dim >= 256); the
residual state stays fp32, with a rounded fp32r copy made per layer for
the GEMM inputs. Out-of-range rows are refreshed with the `oob` vector
between layers via copy_predicated driven by per-core mask/fill inputs,
so all 8 cores run one identical program.
"""

import numpy as np

import concourse.bacc as bacc
import concourse.mybir as mybir
import concourse.tile as tile
from concourse.bass_utils import run_bass_kernel_spmd

N, DIM, NL = 8192, 1024, 4
NCORES = 8
M = N // NCORES           # rows per core
H = 8                     # halo rows each side (sum of dilations)
PAD = 4                   # zero cols so shifted reads stay in-bounds
B = M + 2 * H             # 1040 buffer rows
FB = PAD + B + PAD        # 1048 free-dim cols of the state buffer
DIL = [1, 2, 4, 1]
KT = 3 * DIM // 128       # 24 contraction tiles
DT = DIM // 128           # 8 feature tiles
# Per-layer compute windows (rows [start, start+size) of the B-row buffer),
# shrinking by the dilation each layer; all sizes even (fp32r streams
# column pairs) and >= 256 (fp32r full-rate threshold).
ROW_BLOCKS_L = [
    [(1, 346), (347, 346), (693, 346)],   # layer 1: rows [1, 1039)
    [(3, 346), (349, 344), (693, 344)],   # layer 2: rows [3, 1037)
    [(7, 342), (349, 342), (691, 342)],   # layer 3: rows [7, 1033)
    [(8, 512), (520, 512)],               # layer 4: rows [8, 1032)
]
F32 = mybir.dt.float32
F32R = mybir.dt.float32r

_CACHE = {}
LAST_RESULTS = None  # test harness reads exec_time_ns from here


def _build():
    nc = bacc.Bacc("TRN2", target_bir_lowering=False, debug=False)

    xs_d = nc.dram_tensor("XST", [128, DT, B], F32, kind="ExternalInput")
    w_d = nc.dram_tensor("WT", [NL, 2, KT, 128, 512], F32, kind="ExternalInput")
    b_d = nc.dram_tensor("BS", [128, NL * DT], F32, kind="ExternalInput")
    ml_d = nc.dram_tensor("ML", [128, DT, H], mybir.dt.uint8, kind="ExternalInput")
    fl_d = nc.dram_tensor("FL", [128, DT, H], F32, kind="ExternalInput")
    mr_d = nc.dram_tensor("MR", [128, DT, H], mybir.dt.uint8, kind="ExternalInput")
    fr_d = nc.dram_tensor("FR", [128, DT, H], F32, kind="ExternalInput")
    y_d = nc.dram_tensor("YT", [128, DT, M], F32, kind="ExternalOutput")

    with tile.TileContext(nc) as tc:
        with (
            tc.tile_pool(name="state", bufs=1) as state_pool,
            tc.tile_pool(name="wpool", bufs=1) as w_pool,
            tc.tile_pool(name="const", bufs=1) as const_pool,
            tc.tile_pool(name="tmp", bufs=4) as tmp_pool,
            tc.tile_pool(name="gps", bufs=6, space="PSUM") as gps_pool,
            tc.tile_pool(name="wps", bufs=1, space="PSUM") as wps_pool,
        ):
            S = state_pool.tile([128, DT, FB], F32)    # fp32 residual state
            R = state_pool.tile([128, DT, FB], F32R)   # rounded GEMM input

            # zero the PAD columns once; epilogues never touch them
            nc.gpsimd.memset(S[:, :, 0:PAD], 0.0)
            nc.gpsimd.memset(S[:, :, PAD + B:FB], 0.0)

            # HAM warm-up: the PE idles ~7us in the head waiting for input
            # and weight DMAs. Dummy matmuls there are free (in program
            # order before any real MM) and bring the clock gate to K=8/8
            # so the first real matmuls run at 2.4 GHz instead of 1.2.
            warm_src = const_pool.tile([128, 256], F32)
            warm_w = const_pool.tile([128, 128], F32R)
            warm_rhs = const_pool.tile([128, 256], F32R)
            nc.gpsimd.memset(warm_src[:], 0.0)
            nc.vector.tensor_copy(warm_w[:], warm_src[:, 0:128])
            nc.vector.tensor_copy(warm_rhs[:], warm_src[:])
            warm_ps = wps_pool.tile([128, 256], F32)
            for _ in range(20):
                nc.tensor.matmul(
                    warm_ps[:], warm_w[:], warm_rhs[:], start=True, stop=True
                )

            # ---- entry: host pre-transposed X -> straight DMA into S ----
            # dt0 rides the fast sync queue (it gates the first matmul);
            # the rest go via GpSimd so the weight DMAs own sync
            for dt in range(DT):
                eng = nc.sync if dt == 0 else nc.gpsimd
                eng.dma_start(S[:, dt, PAD:PAD + B], xs_d[:, dt, :])

            bs_t = const_pool.tile([128, NL * DT], F32)
            mask_l = const_pool.tile([128, DT, H], mybir.dt.uint8)
            fill_l = const_pool.tile([128, DT, H], F32)
            mask_r = const_pool.tile([128, DT, H], mybir.dt.uint8)
            fill_r = const_pool.tile([128, DT, H], F32)
            nc.gpsimd.dma_start(bs_t[:], b_d[:])
            nc.gpsimd.dma_start(mask_l[:], ml_d[:])
            nc.gpsimd.dma_start(fill_l[:], fl_d[:])
            nc.gpsimd.dma_start(mask_r[:], mr_d[:])
            nc.gpsimd.dma_start(fill_r[:], fr_d[:])

            # ---- layers ----
            w_tiles = {}
            for l, d in enumerate(DIL):
                # rounded copy of the state for this layer's GEMMs
                for dt in range(DT):
                    nc.vector.tensor_copy(R[:, dt, :], S[:, dt, :])

                row_blocks = ROW_BLOCKS_L[l]
                for h in range(2):
                    for kt in range(KT):
                        wt = w_pool.tile([128, 512], F32R, tag=f"w{h}_{kt}")
                        w_tiles[(h, kt)] = wt
                        nc.sync.dma_start(
                            wt[:], w_d[l, h, kt].bitcast(F32R)
                        )
                    def mm_g(ps, c0, nb, mtl, kt):
                        dt = kt % DT
                        grp = kt // DT
                        sh = 0 if grp == 0 else (-d if grp == 1 else d)
                        nc.tensor.matmul(
                            ps[:, 0:nb],
                            w_tiles[(h, kt)][:, mtl * 128:(mtl + 1) * 128],
                            R[:, dt, PAD + c0 + sh:PAD + c0 + sh + nb],
                            start=(kt == 0),
                            stop=(kt == KT - 1),
                        )

                    def epilogue_g(ps, c0, nb, mtl):
                        mt = h * 4 + mtl
                        tmp = tmp_pool.tile([128, 512], F32, tag="tmp",
                                            name=f"tmp{l}_{h}_{c0}_{mtl}")
                        nc.scalar.activation(
                            tmp[:, 0:nb],
                            ps[:, 0:nb],
                            mybir.ActivationFunctionType.Relu,
                            bias=bs_t[:, l * DT + mt:l * DT + mt + 1],
                            scale=0.5,
                        )
                        # S = 0.5*S + relu(0.5*cat@W + 0.5*b), in place
                        nc.vector.scalar_tensor_tensor(
                            S[:, mt, PAD + c0:PAD + c0 + nb],
                            S[:, mt, PAD + c0:PAD + c0 + nb],
                            0.5,
                            tmp[:, 0:nb],
                            mybir.AluOpType.mult,
                            mybir.AluOpType.add,
                        )

                    groups = [(c0, nb, mtl)
                              for (c0, nb) in row_blocks
                              for mtl in range(4)]
                    # The very first half-layer races the weight DMAs (one
                    # k-tile lands per ~610ns but a single accumulation group
                    # consumes one per ~145ns). Interleave the first 6 groups
                    # per k-tile: consumption 6 x ~145ns/kt stays behind the
                    # supply, so the PE never waits on weights.
                    n_inter = 6 if (l == 0 and h == 0) else 0
                    head = groups[:n_inter]
                    if head:
                        pss = [
                            gps_pool.tile([128, 512], F32, tag="gps",
                                          name=f"psi{i}")
                            for i in range(len(head))
                        ]
                        for kt in range(KT):
                            for i, (c0, nb, mtl) in enumerate(head):
                                mm_g(pss[i], c0, nb, mtl, kt)
                        for i, (c0, nb, mtl) in enumerate(head):
                            epilogue_g(pss[i], c0, nb, mtl)
                    for j, (c0, nb, mtl) in enumerate(groups[n_inter:]):
                        ps = gps_pool.tile([128, 512], F32, tag="gps",
                                           name=f"ps{l}_{h}_{j}")
                        for kt in range(KT):
                            mm_g(ps, c0, nb, mtl, kt)
                        epilogue_g(ps, c0, nb, mtl)

                # refresh out-of-range halo rows with oob (data-driven; only
                # the edge cores have nonzero masks)
                if l < NL - 1:
                    for dt in range(DT):
                        nc.vector.copy_predicated(
                            S[:, dt, PAD:PAD + H],
                            mask_l[:, dt, :], fill_l[:, dt, :],
                        )
                        nc.vector.copy_predicated(
                            S[:, dt, PAD + B - H:PAD + B],
                            mask_r[:, dt, :], fill_r[:, dt, :],
                        )

            # ---- exit: dump the feature-major state; host untransposes ----
            # (two DMAs per dt so each half fires as soon as its last
            # epilogue lands, instead of waiting for the whole row range)
            for dt in range(DT):
                for c0h in (0, 512):
                    nc.sync.dma_start(
                        y_d[:, dt, c0h:c0h + 512],
                        S[:, dt, PAD + H + c0h:PAD + H + c0h + 512],
                    )

    nc.compile()
    return nc


def _get_nc():
    if "nc" not in _CACHE:
        _CACHE["nc"] = _build()
    return _CACHE["nc"]


def kernel(X, Ws, bs, oob):
    global LAST_RESULTS
    X = np.ascontiguousarray(np.asarray(X, np.float32))
    Ws = np.ascontiguousarray(np.asarray(Ws, np.float32))
    bs = np.ascontiguousarray(np.asarray(bs, np.float32))
    oob = np.ascontiguousarray(np.asarray(oob, np.float32))

    nc = _get_nc()

    # host-side input prep (pure layout rearrangement)
    WT = np.ascontiguousarray(
        Ws.reshape(NL, KT, 128, 2, 512).transpose(0, 3, 1, 2, 4)
    )
    BS = np.ascontiguousarray(
        (0.5 * bs).reshape(NL, DT, 128).transpose(2, 0, 1).reshape(128, NL * DT)
    )
    oobT = np.ascontiguousarray(oob.reshape(DT, 128).T)  # [128, DT]
    fill_edge = np.repeat(oobT[:, :, None], H, axis=2)   # [128, DT, H]
    ones = np.ones((128, DT, H), np.uint8)
    zeros_m = np.zeros((128, DT, H), np.uint8)
    zeros = np.zeros((128, DT, H), np.float32)

    in_maps = []
    for c in range(NCORES):
        lo, hi = c * M - H, c * M + M + H
        xs = np.empty((B, DIM), np.float32)
        slo, shi = max(lo, 0), min(hi, N)
        xs[slo - lo:shi - lo] = X[slo:shi]
        if lo < 0:
            xs[0:-lo] = oob
        if hi > N:
            xs[B - (hi - N):] = oob
        xst = np.ascontiguousarray(
            xs.reshape(B, DT, 128).transpose(2, 1, 0))
        left_edge = c == 0
        right_edge = c == NCORES - 1
        in_maps.append({
            "XST": xst,
            "WT": WT,
            "BS": BS,
            "ML": ones if left_edge else zeros_m,
            "FL": fill_edge if left_edge else zeros,
            "MR": ones if right_edge else zeros_m,
            "FR": fill_edge if right_edge else zeros,
        })

    res = run_bass_kernel_spmd(nc, in_maps, list(range(NCORES)))
    LAST_RESULTS = res
    out = np.concatenate(
        [res.results[c]["YT"].transpose(2, 1, 0).reshape(M, DIM)
         for c in range(NCORES)],
        axis=0,
    )
    return out[None, :, :].astype(np.float32)


# revision 43
# speedup vs baseline: 1.0089x; 1.0022x over previous
"""DilatedCNN forward on 8 TRN2 NeuronCores.

Strategy: data-parallel over the sequence dim N with halo. Each core owns
M=1024 rows plus an 8-row halo on each side (8 = sum of dilations
[1,2,4,1]); with the halo, all four layers are computed fully locally —
no collectives. The activation state lives in SBUF *transposed*
(feature-major: [128 partitions = feature chunk, rows in the free dim]) so
that
  * the concat [X, X_left, X_right] is just three column-shifted views of
    the same buffer (shifts along the free dim are free),
  * the 3072-feature contraction has features on partitions as the
    TensorEngine requires for both operands,
  * each layer's output is again feature-major — ready to be the next
    layer's input with no data movement,
  * the per-feature bias is a per-partition scalar for the activation op.
Matmuls run in float32r (TF32-path, full PE rate at free-dim >= 256); the
residual state stays fp32, with a rounded fp32r copy made per layer for
the GEMM inputs. Out-of-range rows are refreshed with the `oob` vector
between layers via copy_predicated driven by per-core mask/fill inputs,
so all 8 cores run one identical program.
"""

import numpy as np

import concourse.bacc as bacc
import concourse.mybir as mybir
import concourse.tile as tile
from concourse.bass_utils import run_bass_kernel_spmd

N, DIM, NL = 8192, 1024, 4
NCORES = 8
M = N // NCORES           # rows per core
H = 8                     # halo rows each side (sum of dilations)
PAD = 4                   # zero cols so shifted reads stay in-bounds
B = M + 2 * H             # 1040 buffer rows
FB = PAD + B + PAD        # 1048 free-dim cols of the state buffer
DIL = [1, 2, 4, 1]
KT = 3 * DIM // 128       # 24 contraction tiles
DT = DIM // 128           # 8 feature tiles
# Per-layer compute windows (rows [start, start+size) of the B-row buffer),
# shrinking by the dilation each layer; all sizes even (fp32r streams
# column pairs) and >= 256 (fp32r full-rate threshold).
ROW_BLOCKS_L = [
    [(1, 346), (347, 346), (693, 346)],   # layer 1: rows [1, 1039)
    [(3, 346), (349, 344), (693, 344)],   # layer 2: rows [3, 1037)
    [(7, 342), (349, 342), (691, 342)],   # layer 3: rows [7, 1033)
    [(8, 512), (520, 512)],               # layer 4: rows [8, 1032)
]
F32 = mybir.dt.float32
F32R = mybir.dt.float32r

_CACHE = {}
LAST_RESULTS = None  # test harness reads exec_time_ns from here


def _build():
    nc = bacc.Bacc("TRN2", target_bir_lowering=False, debug=False)

    xs_d = nc.dram_tensor("XST", [128, DT, B], F32, kind="ExternalInput")
    w_d = nc.dram_tensor("WT", [NL, 2, KT, 128, 512], F32, kind="ExternalInput")
    b_d = nc.dram_tensor("BS", [128, NL * DT], F32, kind="ExternalInput")
    ml_d = nc.dram_tensor("ML", [128, DT, H], mybir.dt.uint8, kind="ExternalInput")
    fl_d = nc.dram_tensor("FL", [128, DT, H], F32, kind="ExternalInput")
    mr_d = nc.dram_tensor("MR", [128, DT, H], mybir.dt.uint8, kind="ExternalInput")
    fr_d = nc.dram_tensor("FR", [128, DT, H], F32, kind="ExternalInput")
    y_d = nc.dram_tensor("YT", [128, DT, M], F32, kind="ExternalOutput")

    with tile.TileContext(nc) as tc:
        with (
            tc.tile_pool(name="state", bufs=1) as state_pool,
            tc.tile_pool(name="wpool", bufs=1) as w_pool,
            tc.tile_pool(name="const", bufs=1) as const_pool,
            tc.tile_pool(name="tmp", bufs=4) as tmp_pool,
            tc.tile_pool(name="gps", bufs=6, space="PSUM") as gps_pool,
            tc.tile_pool(name="wps", bufs=1, space="PSUM") as wps_pool,
        ):
            S = state_pool.tile([128, DT, FB], F32)    # fp32 residual state
            R = state_pool.tile([128, DT, FB], F32R)   # rounded GEMM input

            # zero the PAD columns once; epilogues never touch them
            nc.gpsimd.memset(S[:, :, 0:PAD], 0.0)
            nc.gpsimd.memset(S[:, :, PAD + B:FB], 0.0)

            # HAM warm-up: the PE idles ~7us in the head waiting for input
            # and weight DMAs. Dummy matmuls there are free (in program
            # order before any real MM) and bring the clock gate to K=8/8
            # so the first real matmuls run at 2.4 GHz instead of 1.2.
            warm_src = const_pool.tile([128, 256], F32)
            warm_w = const_pool.tile([128, 128], F32R)
            warm_rhs = const_pool.tile([128, 256], F32R)
            nc.gpsimd.memset(warm_src[:], 0.0)
            nc.vector.tensor_copy(warm_w[:], warm_src[:, 0:128])
            nc.vector.tensor_copy(warm_rhs[:], warm_src[:])
            warm_ps = wps_pool.tile([128, 256], F32)
            for _ in range(10):
                nc.tensor.matmul(
                    warm_ps[:], warm_w[:], warm_rhs[:], start=True, stop=True
                )

            # ---- entry: host pre-transposed X -> straight DMA into S ----
            # dt0 gates the first matmul: split it into 4 chunks so the
            # transfer spreads across parallel HW queues (~0.9us vs ~3.5us);
            # the rest go via GpSimd so the weight DMAs own sync
            for q in range(4):
                nc.sync.dma_start(
                    S[:, 0, PAD + q * 260:PAD + (q + 1) * 260],
                    xs_d[:, 0, q * 260:(q + 1) * 260],
                )
            for dt in range(1, DT):
                nc.gpsimd.dma_start(S[:, dt, PAD:PAD + B], xs_d[:, dt, :])

            bs_t = const_pool.tile([128, NL * DT], F32)
            mask_l = const_pool.tile([128, DT, H], mybir.dt.uint8)
            fill_l = const_pool.tile([128, DT, H], F32)
            mask_r = const_pool.tile([128, DT, H], mybir.dt.uint8)
            fill_r = const_pool.tile([128, DT, H], F32)
            nc.gpsimd.dma_start(bs_t[:], b_d[:])
            nc.gpsimd.dma_start(mask_l[:], ml_d[:])
            nc.gpsimd.dma_start(fill_l[:], fl_d[:])
            nc.gpsimd.dma_start(mask_r[:], mr_d[:])
            nc.gpsimd.dma_start(fill_r[:], fr_d[:])

            # ---- layers ----
            w_tiles = {}
            for l, d in enumerate(DIL):
                # rounded copy of the state for this layer's GEMMs
                for dt in range(DT):
                    nc.vector.tensor_copy(R[:, dt, :], S[:, dt, :])

                row_blocks = ROW_BLOCKS_L[l]
                for h in range(2):
                    for kt in range(KT):
                        wt = w_pool.tile([128, 512], F32R, tag=f"w{h}_{kt}")
                        w_tiles[(h, kt)] = wt
                        nc.sync.dma_start(
                            wt[:], w_d[l, h, kt].bitcast(F32R)
                        )
                    def mm_g(ps, c0, nb, mtl, kt):
                        dt = kt % DT
                        grp = kt // DT
                        sh = 0 if grp == 0 else (-d if grp == 1 else d)
                        nc.tensor.matmul(
                            ps[:, 0:nb],
                            w_tiles[(h, kt)][:, mtl * 128:(mtl + 1) * 128],
                            R[:, dt, PAD + c0 + sh:PAD + c0 + sh + nb],
                            start=(kt == 0),
                            stop=(kt == KT - 1),
                        )

                    def epilogue_g(ps, c0, nb, mtl):
                        mt = h * 4 + mtl
                        tmp = tmp_pool.tile([128, 512], F32, tag="tmp",
                                            name=f"tmp{l}_{h}_{c0}_{mtl}")
                        nc.scalar.activation(
                            tmp[:, 0:nb],
                            ps[:, 0:nb],
                            mybir.ActivationFunctionType.Relu,
                            bias=bs_t[:, l * DT + mt:l * DT + mt + 1],
                            scale=0.5,
                        )
                        # S = 0.5*S + relu(0.5*cat@W + 0.5*b), in place
                        nc.vector.scalar_tensor_tensor(
                            S[:, mt, PAD + c0:PAD + c0 + nb],
                            S[:, mt, PAD + c0:PAD + c0 + nb],
                            0.5,
                            tmp[:, 0:nb],
                            mybir.AluOpType.mult,
                            mybir.AluOpType.add,
                        )

                    groups = [(c0, nb, mtl)
                              for (c0, nb) in row_blocks
                              for mtl in range(4)]
                    # The very first half-layer races the weight DMAs (one
                    # k-tile lands per ~610ns but a single accumulation group
                    # consumes one per ~145ns). Interleave the first 6 groups
                    # per k-tile: consumption 6 x ~145ns/kt stays behind the
                    # supply, so the PE never waits on weights.
                    n_inter = 6 if (l == 0 and h == 0) else 0
                    head = groups[:n_inter]
                    if head:
                        pss = [
                            gps_pool.tile([128, 512], F32, tag="gps",
                                          name=f"psi{i}")
                            for i in range(len(head))
                        ]
                        for kt in range(KT):
                            for i, (c0, nb, mtl) in enumerate(head):
                                mm_g(pss[i], c0, nb, mtl, kt)
                        for i, (c0, nb, mtl) in enumerate(head):
                            epilogue_g(pss[i], c0, nb, mtl)
                    for j, (c0, nb, mtl) in enumerate(groups[n_inter:]):
                        ps = gps_pool.tile([128, 512], F32, tag="gps",
                                           name=f"ps{l}_{h}_{j}")
                        for kt in range(KT):
                            mm_g(ps, c0, nb, mtl, kt)
                        epilogue_g(ps, c0, nb, mtl)

                # refresh out-of-range halo rows with oob (data-driven; only
                # the edge cores have nonzero masks)
                if l < NL - 1:
                    for dt in range(DT):
                        nc.vector.copy_predicated(
                            S[:, dt, PAD:PAD + H],
                            mask_l[:, dt, :], fill_l[:, dt, :],
                        )
                        nc.vector.copy_predicated(
                            S[:, dt, PAD + B - H:PAD + B],
                            mask_r[:, dt, :], fill_r[:, dt, :],
                        )

            # ---- exit: dump the feature-major state; host untransposes ----
            # (two DMAs per dt so each half fires as soon as its last
            # epilogue lands, instead of waiting for the whole row range)
            for dt in range(DT):
                for c0h in (0, 512):
                    nc.sync.dma_start(
                        y_d[:, dt, c0h:c0h + 512],
                        S[:, dt, PAD + H + c0h:PAD + H + c0h + 512],
                    )

    nc.compile()
    return nc


def _get_nc():
    if "nc" not in _CACHE:
        _CACHE["nc"] = _build()
    return _CACHE["nc"]


def kernel(X, Ws, bs, oob):
    global LAST_RESULTS
    X = np.ascontiguousarray(np.asarray(X, np.float32))
    Ws = np.ascontiguousarray(np.asarray(Ws, np.float32))
    bs = np.ascontiguousarray(np.asarray(bs, np.float32))
    oob = np.ascontiguousarray(np.asarray(oob, np.float32))

    nc = _get_nc()

    # host-side input prep (pure layout rearrangement)
    WT = np.ascontiguousarray(
        Ws.reshape(NL, KT, 128, 2, 512).transpose(0, 3, 1, 2, 4)
    )
    BS = np.ascontiguousarray(
        (0.5 * bs).reshape(NL, DT, 128).transpose(2, 0, 1).reshape(128, NL * DT)
    )
    oobT = np.ascontiguousarray(oob.reshape(DT, 128).T)  # [128, DT]
    fill_edge = np.repeat(oobT[:, :, None], H, axis=2)   # [128, DT, H]
    ones = np.ones((128, DT, H), np.uint8)
    zeros_m = np.zeros((128, DT, H), np.uint8)
    zeros = np.zeros((128, DT, H), np.float32)

    in_maps = []
    for c in range(NCORES):
        lo, hi = c * M - H, c * M + M + H
        xs = np.empty((B, DIM), np.float32)
        slo, shi = max(lo, 0), min(hi, N)
        xs[slo - lo:shi - lo] = X[slo:shi]
        if lo < 0:
            xs[0:-lo] = oob
        if hi > N:
            xs[B - (hi - N):] = oob
        xst = np.ascontiguousarray(
            xs.reshape(B, DT, 128).transpose(2, 1, 0))
        left_edge = c == 0
        right_edge = c == NCORES - 1
        in_maps.append({
            "XST": xst,
            "WT": WT,
            "BS": BS,
            "ML": ones if left_edge else zeros_m,
            "FL": fill_edge if left_edge else zeros,
            "MR": ones if right_edge else zeros_m,
            "FR": fill_edge if right_edge else zeros,
        })

    res = run_bass_kernel_spmd(nc, in_maps, list(range(NCORES)))
    LAST_RESULTS = res
    out = np.concatenate(
        [res.results[c]["YT"].transpose(2, 1, 0).reshape(M, DIM)
         for c in range(NCORES)],
        axis=0,
    )
    return out[None, :, :].astype(np.float32)
